# revision 48
# baseline (speedup 1.0000x reference)
"""Trainium2 Bass kernel for nn_Causal_Attention_13082470383895.

Full (unsharded) inputs in, full output out. Internally shards batch*heads
across 8 NeuronCores: core c owns batch c//4 and the 4 heads [4*(c%4), 4*(c%4)+4).
Each core computes its heads' q/k/v projections (column-sharded weights),
QK-layernorm, causal unnormalized-exp attention, and its partial contribution
to the output projection (row-sharded W_out). Host sums the 4 partials per batch.

All matmul operands are bf16 (PSUM accumulates fp32). x is pre-transposed and
cast to bf16 on the host, so the kernel needs no x transposes. Output partials
are bf16; the host sums them in fp32.

Schedule: the attention phase for supertile s is ACT(exp)-bound while the
projection/layernorm phase is PE-bound, so phase_a(s+1) and phase_c(s-1) are
emitted as PE "filler" interleaved between attention items — the PE works on
projections while the ACT catches up on exp, and AV(j) runs in the shadow of
QK(j+1)+filler. The softmax tail is split: the PSUM reads (av/den copies) are
emitted immediately (before the av PSUM slot is recycled), the
reciprocal/broadcast/normalize two items later so the PE never waits on the
reciprocal. reciprocal_approx_fast needs a partition-0 input AP (the custom
uops corrupt data at a nonzero partition base on HW).

Hardcoded shapes (per spec): inputs [2, 2048, 1024], W_qk [1024, 2048],
W_v [1024, 1024], W_out [1024, 1024], q/k scale=ones, bias=zeros (per spec
fill; layernorm affine is identity and is not applied).
"""

import math
import os
import sys

import numpy as np

sys.path.insert(0, "/opt/trn_rl_repo")

B = 2
L = 2048
D = 1024
HEADS = 16
DIM = 64
LN_EPS = 1e-6
P = 128
LT = L // P          # 16 l-tiles
DT = D // P          # 8 contraction tiles
NHL = 4              # heads per core
SUP = 4              # 512-wide l_q supertiles
N_CORES = 8

_CACHE = {}


def _make_bacc_cls():
    import bass_rust
    import concourse.mybir as mybir
    from concourse import bacc
    from concourse.hw_specs import get_activation_tables

    class KernelBacc(bacc.Bacc):
        """Bacc whose ACT-table selector never picks the `natural_log` set
        for Ln: hiding `ln` there makes the greedy selector choose
        `natural_log_exp_and_others` (which also holds exp/copy/identity/
        square), so the kernel needs a single table load."""

        def insert_act_table_loads(self):
            has_activation = any(
                isinstance(i, mybir.InstActivation)
                for b in self.main_func.blocks
                for i in b.instructions
            )
            if not has_activation:
                return
            ln = mybir.ActivationFunctionType.Ln
            tables = []
            for name, funcs in get_activation_tables(self.m.arch).items():
                if name == "natural_log":
                    funcs = funcs - {ln}
                tables.append((name, funcs))
            bass_rust.insert_act_table_loads(self, tables)

    return KernelBacc


def _build_nc():
    import concourse.bass as bass
    import concourse.mybir as mybir
    import concourse.tile as tile
    from concourse.masks import make_identity, make_upper_triangular

    f32 = mybir.dt.float32
    bf16 = mybir.dt.bfloat16
    AF = mybir.ActivationFunctionType
    ALU = mybir.AluOpType
    AX = mybir.AxisListType

    # layernorm over raw (unscaled) qk groups of 64:
    # (raw - m) / sqrt(var_raw + 1024*eps)  with  m2 = 64*var_raw
    #   rstd = 8 / sqrt(m2 + 64*1024*eps) = exp(-0.5*ln(m2 + EPS2) + ln 8)
    EPS2 = float(DIM * D * LN_EPS)      # 0.065536
    LN8 = float(math.log(8.0))

    nc = _make_bacc_cls()("TRN2", target_bir_lowering=False, debug=False)

    XT = nc.dram_tensor("xt", [D, L], bf16, kind="ExternalInput").ap()
    WQK = nc.dram_tensor("w_qk", [D, 512], bf16, kind="ExternalInput").ap()
    WV = nc.dram_tensor("w_v", [D, 256], bf16, kind="ExternalInput").ap()
    WOUT = nc.dram_tensor("w_out", [256, D], bf16, kind="ExternalInput").ap()
    OUT = nc.dram_tensor("out", [L, D], bf16, kind="ExternalOutput").ap()

    with tile.TileContext(nc) as tc:
        const = tc.alloc_tile_pool(name="const", bufs=1)
        big = tc.alloc_tile_pool(name="big", bufs=1)
        work = tc.alloc_tile_pool(name="work", bufs=2)
        stat = tc.alloc_tile_pool(name="stat", bufs=3)
        esp = tc.alloc_tile_pool(name="esp", bufs=6)
        outp = tc.alloc_tile_pool(name="outp", bufs=4)

        ident = const.tile([P, P], bf16)
        make_identity(nc, ident)
        up01 = const.tile([P, P], bf16)
        make_upper_triangular(nc, up01, val=1.0, diag=True)
        ones_row = const.tile([1, DIM], bf16)
        nc.vector.memset(ones_row, 1.0)
        eps2b = const.tile([P, 1], f32)
        nc.vector.memset(eps2b, EPS2)
        ln8b = const.tile([P, 1], f32)
        nc.vector.memset(ln8b, LN8)

        tc.strict_bb_all_engine_barrier()

        # DMAs after the barrier (issuing before it delays every engine by
        # the serial issue time; measured ~10us later first matmul). wqk
        # before xt — the first projection needs all of wqk but only xt
        # chunk 0; wv/wout on the ACT hwdge queue in parallel.
        wqk = big.tile([P, DT, 512], bf16)
        nc.sync.dma_start(wqk, WQK.rearrange("(c p) n -> p c n", p=P))
        xt = big.tile([P, DT, L], bf16)
        xt_src = XT.rearrange("(c p) l -> p c l", p=P)
        for c in range(DT):
            nc.sync.dma_start(xt[:, c], xt_src[:, c])
        wv = big.tile([P, DT, 256], bf16)
        nc.scalar.dma_start(wv, WV.rearrange("(c p) n -> p c n", p=P))
        wout = big.tile([P, 2, D], bf16)
        nc.scalar.dma_start(wout, WOUT.rearrange("(c p) n -> p c n", p=P))

        v_sb = big.tile([P, LT, NHL, DIM + 1], bf16)
        qt = [big.tile([P, L], bf16, name=f"qt{i}") for i in range(2)]
        kt = [big.tile([P, L], bf16, name=f"kt{i}") for i in range(2)]
        at = [big.tile([P, L], bf16, name=f"at{i}") for i in range(2)]
        nc.vector.memset(v_sb[:, :, :, DIM], 1.0)

        # PSUM layout: "st" (attention scores) 3x2KB, "pa" (projection /
        # transpose / broadcast) 3x2KB, "op" (av accumulators + out-proj) 4KB.
        with tc.tile_pool(name="ps", bufs=3, space="PSUM") as ps:

            # ---------- phase_a / phase_c emission units ----------

            def unit_qk_proj(t, chalf):
                """Half of one l-tile's qk projection (4 contraction mms)."""
                tsl = slice(t * P, (t + 1) * P)
                if chalf == 0:
                    qk_ps = ps.tile([P, 512], f32, tag="pa", bufs=2,
                                    name="qk_ps")
                    unit_qk_proj.live[t] = qk_ps
                else:
                    qk_ps = unit_qk_proj.live[t]
                for c in range(4 * chalf, 4 * chalf + 4):
                    nc.tensor.matmul(
                        qk_ps, xt[:, c, tsl], wqk[:, c],
                        start=(c == 0), stop=(c == DT - 1),
                    )

            unit_qk_proj.live = {}

            def unit_ln(t):
                """Layernorm chain for tile t; returns bf16 normalized qn."""
                qk_ps = unit_qk_proj.live.pop(t)
                qk3 = qk_ps.rearrange("p (g d) -> p g d", g=8)
                sum_ = stat.tile([P, 8], f32, tag="sum")
                nc.vector.tensor_reduce(sum_, qk3, AX.X, ALU.add)
                sq = work.tile([P, 8, DIM], f32, tag="sq", bufs=2)
                nc.scalar.activation(sq, qk3, AF.Square)
                ssq = stat.tile([P, 8], f32, tag="ssq")
                nc.vector.tensor_reduce(ssq, sq, AX.X, ALU.add)
                t1 = stat.tile([P, 8], f32, tag="t1")
                nc.vector.tensor_tensor(t1, sum_, sum_, ALU.mult)
                m2 = stat.tile([P, 8], f32, tag="m2")
                nc.vector.scalar_tensor_tensor(
                    m2, t1, -1.0 / DIM, ssq, op0=ALU.mult, op1=ALU.add)
                rstd = stat.tile([P, 8], f32, tag="rstd")
                nc.scalar.activation(rstd, m2, AF.Ln, bias=eps2b, scale=1.0)
                nc.scalar.activation(rstd, rstd, AF.Exp, bias=ln8b,
                                     scale=-0.5)
                negprod = stat.tile([P, 8], f32, tag="negprod")
                nc.vector.scalar_tensor_tensor(
                    negprod, sum_, -1.0 / DIM, rstd,
                    op0=ALU.mult, op1=ALU.mult)
                qn = work.tile([P, 8, DIM], bf16, tag="qn", bufs=6, name="qn")
                # normalize split across ACT (groups 0-3) and DVE (4-7)
                for g in range(4):
                    nc.scalar.activation(
                        qn[:, g], qk3[:, g], AF.Identity,
                        bias=negprod[:, g:g + 1], scale=rstd[:, g:g + 1])
                for g in range(4, 8):
                    nc.vector.tensor_scalar(
                        qn[:, g], qk3[:, g],
                        rstd[:, g:g + 1], negprod[:, g:g + 1],
                        op0=ALU.mult, op1=ALU.add)
                return qn

            def unit_v_proj(t, chalf):
                tsl = slice(t * P, (t + 1) * P)
                if chalf == 0:
                    v_ps = ps.tile([P, 512], f32, tag="pa", bufs=2,
                                   name="v_ps")
                    unit_v_proj.live[t] = v_ps
                else:
                    v_ps = unit_v_proj.live[t]
                for c in range(4 * chalf, 4 * chalf + 4):
                    nc.tensor.matmul(
                        v_ps[:, :256], xt[:, c, tsl], wv[:, c],
                        start=(c == 0), stop=(c == DT - 1),
                    )
                if chalf == 1:
                    nc.vector.tensor_copy(
                        v_sb[:, t, :, :DIM],
                        v_ps[:, :256].rearrange("p (h d) -> p h d", h=NHL))
                    del unit_v_proj.live[t]

            unit_v_proj.live = {}

            def unit_transpose(s, qn_tiles, hl, which):
                pr, ro = hl // 2, DIM * (hl % 2)
                dst = qt if which == 0 else kt
                tp_ps = ps.tile([DIM, 512], bf16, tag="pa", bufs=2,
                                name="tp_ps")
                for i in range(4):
                    nc.tensor.transpose(
                        tp_ps[:, i * P:(i + 1) * P],
                        qn_tiles[i][:, 2 * hl + which],
                        ident,
                    )
                nc.vector.tensor_copy(
                    dst[pr][ro:ro + DIM, s * 512:(s + 1) * 512], tp_ps)

            def unit_out_proj(t):
                # shares the "st" slots (same 4KB size) — the "op" tag is
                # reserved for the attention av accumulators, so out-proj
                # units can interleave into the attention stream.
                op_ps = ps.tile([P, D], f32, tag="st", bufs=2, name="op_ps")
                for nch in range(2):
                    for c in range(2):
                        nc.tensor.matmul(
                            op_ps[:, nch * 512:(nch + 1) * 512],
                            at[c][:, t * P:(t + 1) * P],
                            wout[:, c, nch * 512:(nch + 1) * 512],
                            start=(c == 0), stop=(c == 1),
                        )
                o_sb = outp.tile([P, D], bf16, tag="o")
                # 1/32 (v proj) * 1/32 (out proj) = 1/1024; on DVE — the
                # ACT engine is the attention bottleneck
                nc.vector.tensor_scalar(
                    o_sb, op_ps, 1.0 / 1024.0, None, op0=ALU.mult)
                # two half-width stores land on different DMA queues, halving
                # the per-store drain (~11us for 256KB on one queue) that
                # otherwise dominates the kernel tail
                nc.sync.dma_start(OUT[t * P:(t + 1) * P, :512],
                                  o_sb[:, :512])
                nc.sync.dma_start(OUT[t * P:(t + 1) * P, 512:],
                                  o_sb[:, 512:])

            def unit_qk_proj_pair(t0, t1, c):
                """Chunk-outer qk projection for a pair of l-tiles: one
                contraction chunk for both tiles, so supertile 0's matmuls
                pace with the xt chunk DMAs instead of waiting for all 8."""
                for t in (t0, t1):
                    if c == 0:
                        unit_qk_proj.live[t] = ps.tile(
                            [P, 512], f32, tag="pa", bufs=2, name="qk_ps")
                    tsl = slice(t * P, (t + 1) * P)
                    nc.tensor.matmul(
                        unit_qk_proj.live[t], xt[:, c, tsl], wqk[:, c],
                        start=(c == 0), stop=(c == DT - 1),
                    )

            def gen_phase_a_units(s):
                """Yield emission thunks for supertile s's projections/LN/
                transposes, in dependency-friendly order."""
                qn_tiles = []

                def ln_unit(t):
                    def go():
                        qn_tiles.append(unit_ln(t))
                    return go

                if s == 0:
                    for pair in range(2):
                        t0, t1 = 2 * pair, 2 * pair + 1
                        for c in range(DT):
                            yield (lambda t0=t0, t1=t1, c=c:
                                   unit_qk_proj_pair(t0, t1, c))
                        yield ln_unit(t0)
                        yield ln_unit(t1)
                else:
                    for t in range(4 * s, 4 * s + 4):
                        yield lambda t=t: unit_qk_proj(t, 0)
                        yield lambda t=t: unit_qk_proj(t, 1)
                        yield ln_unit(t)
                for t in range(4 * s, 4 * s + 4):
                    yield lambda t=t: unit_v_proj(t, 0)
                    yield lambda t=t: unit_v_proj(t, 1)
                for hl in range(NHL):
                    for which in range(2):
                        yield (lambda hl=hl, which=which:
                               unit_transpose(s, qn_tiles, hl, which))

            def gen_phase_c_units(s):
                for t in range(4 * s, 4 * s + 4):
                    yield lambda t=t: unit_out_proj(t)

            # ---------- attention ----------

            def phase_b(s, filler):
                """Attention for supertile s; `filler` is a list of thunks
                (next supertile's projections + previous out-projection)
                drained evenly across the attention items as PE filler."""
                ls = slice(s * 512, (s + 1) * 512)
                njs = 4 * s + 4
                av_pair = {}

                def issue_qk_exp(pr, j):
                    pp = j - 4 * s
                    woff = max(0, pp) * P
                    st_pair = ps.tile([P, 2, 512], f32, tag="st", bufs=2,
                                      name="st_pair")
                    for r01 in range(2):
                        ro = DIM * r01
                        nc.tensor.matmul(
                            st_pair[:, r01],
                            kt[pr][ro:ro + DIM, j * P:(j + 1) * P],
                            qt[pr][ro:ro + DIM, ls],
                            start=True, stop=True, tile_position=(ro, 0),
                        )
                    # one wide exp for both heads (halves the ACT op count)
                    es_pair = esp.tile([P, 2, 512], bf16, tag="es", bufs=3)
                    nc.scalar.activation(es_pair[:, :, woff:],
                                         st_pair[:, :, woff:],
                                         AF.Exp, scale=1.0 / DIM)
                    if pp >= 0:
                        blk = slice(pp * P, (pp + 1) * P)
                        nc.vector.tensor_tensor(
                            es_pair[:, :, blk], es_pair[:, :, blk],
                            up01.rearrange("p (o x) -> p o x",
                                           o=1).to_broadcast([P, 2, P]),
                            ALU.mult)
                    return woff, es_pair

                def issue_av(pr, j, woff, es_pair):
                    if j == 0:
                        av_pair[pr] = ps.tile([DIM + 1, 1024], f32, tag="op",
                                              bufs=1, name="av_pair")
                    for r01 in range(2):
                        hl = 2 * pr + r01
                        nc.tensor.matmul(
                            av_pair[pr][:, r01 * 512 + woff:(r01 + 1) * 512],
                            v_sb[:, j, hl],
                            es_pair[:, r01, woff:],
                            start=(j == 0), stop=(j == njs - 1),
                        )

                def tail_copies(pr, av):
                    """PSUM reads — must be emitted before the av slot is
                    recycled by the next op-tag allocation."""
                    out = []
                    for r01 in range(2):
                        vsl = slice(r01 * 512, (r01 + 1) * 512)
                        av_sb = esp.tile([DIM, 512], bf16, tag="avsb",
                                         bufs=4)
                        nc.vector.tensor_copy(av_sb, av[:DIM, vsl])
                        den_sb = stat.tile([1, 512], f32, tag="den")
                        nc.vector.tensor_copy(den_sb, av[DIM:DIM + 1, vsl])
                        out.append((av_sb, den_sb))
                    return out

                def tail_norm(pr, staged):
                    for r01 in range(2):
                        ro = DIM * r01
                        av_sb, den_sb = staged[r01]
                        recip = stat.tile([1, 512], f32, tag="recip")
                        nc.vector.reciprocal_approx_fast(recip, den_sb)
                        recip_bf = stat.tile([1, 512], bf16, tag="recipb")
                        nc.vector.tensor_copy(recip_bf, recip)
                        bc_ps = ps.tile([DIM, 512], f32, tag="pa", bufs=2,
                                        name="bc_ps")
                        nc.tensor.matmul(bc_ps, ones_row, recip_bf,
                                         start=True, stop=True)
                        nc.vector.tensor_tensor(at[pr][ro:ro + DIM, ls],
                                                av_sb, bc_ps, ALU.mult)

                flat = [(pr, j) for pr in range(2) for j in range(njs)]
                pend = []
                norm_q = []
                n_items = len(flat)
                fill_left = list(filler)

                def emit_fill(i):
                    k = -(-len(fill_left) // max(1, n_items - i))  # ceil
                    for _ in range(min(k, len(fill_left))):
                        fill_left.pop(0)()

                def step_norms():
                    for ent in list(norm_q):
                        if ent[2] <= 0:
                            tail_norm(ent[0], ent[1])
                            norm_q.remove(ent)
                        else:
                            ent[2] -= 1

                def pop_av():
                    pr0, j0, woff0, es0 = pend.pop(0)
                    issue_av(pr0, j0, woff0, es0)
                    if j0 == njs - 1:
                        staged = tail_copies(pr0, av_pair.pop(pr0))
                        norm_q.append([pr0, staged, 2])

                for i, (pr, j) in enumerate(flat):
                    pend.append((pr, j, *issue_qk_exp(pr, j)))
                    emit_fill(i)
                    step_norms()
                    if len(pend) > 1:
                        pop_av()
                while pend:
                    pop_av()
                    step_norms()
                for _ in range(3):
                    step_norms()
                for f in fill_left:
                    f()

            # ---------- main schedule ----------

            # supertile 0's projections run standalone (nothing to overlap)
            for f in gen_phase_a_units(0):
                f()
            for s in range(SUP):
                filler = list(gen_phase_a_units(s + 1)) if s + 1 < SUP else []
                if s > 0:
                    filler.extend(gen_phase_c_units(s - 1))
                phase_b(s, filler)
            for f in gen_phase_c_units(SUP - 1):
                f()

        outp.release()
        esp.release()
        stat.release()
        work.release()
        big.release()
        const.release()

    nc.finalize()
    return nc


def _get_nc():
    if "nc" not in _CACHE:
        _CACHE["nc"] = _build_nc()
    return _CACHE["nc"]


def kernel(**inputs):
    import ml_dtypes

    bf16 = ml_dtypes.bfloat16
    x = np.asarray(inputs["inputs"], dtype=np.float32)
    w_qk = np.asarray(inputs["W_qk"], dtype=np.float32)
    w_v = np.asarray(inputs["W_v"], dtype=np.float32)
    w_out = np.asarray(inputs["W_out"], dtype=np.float32)

    xt = [np.ascontiguousarray(x[b].T).astype(bf16) for b in range(B)]

    nc = _get_nc()
    in_maps = []
    for c in range(N_CORES):
        b, g = divmod(c, 4)
        in_maps.append({
            "xt": xt[b],
            "w_qk": np.ascontiguousarray(
                w_qk[:, 512 * g:512 * (g + 1)]).astype(bf16),
            "w_v": np.ascontiguousarray(
                w_v[:, 256 * g:256 * (g + 1)]).astype(bf16),
            "w_out": np.ascontiguousarray(
                w_out[256 * g:256 * (g + 1), :]).astype(bf16),
        })

    from concourse.bass_utils import run_bass_kernel_spmd

    trace = bool(os.environ.get("KERNEL_TRACE"))
    if trace:
        try:
            from antenv.axon_hooks import get_axon_ntff_profile_hook
            if get_axon_ntff_profile_hook() is None:
                trace = False
        except Exception:
            trace = False
    res = run_bass_kernel_spmd(nc, in_maps, core_ids=list(range(N_CORES)),
                               trace=trace)
    _CACHE["last_results"] = res
    outs = [m["out"].astype(np.float32) for m in res.results]
    out = np.stack([
        outs[0] + outs[1] + outs[2] + outs[3],
        outs[4] + outs[5] + outs[6] + outs[7],
    ]).astype(np.float32)
    return out
